# revision 9
# baseline (speedup 1.0000x reference)
"""Causal GQA self-attention block (B=4, T=2048, C=1024, H=16, G=4) on 8
Trainium2 NeuronCores.

Sharding: core c = d*4+g  (d in {0,1} batch-DP, g in {0..3} kv-group TP).
Each core handles batches [2d, 2d+1], heads {g, g+4, g+8, g+12}, kv group g,
and produces a partial projection output; the host sums the 4 TP partials
per batch pair and adds the bias.

Per-core kernel (all matmuls bf16, fp32 PSUM accumulation):
  - fused QKV projection from pre-transposed x (host supplies x^T),
    producing Q^T / K^T / V^T with channels on partitions
  - scores computed transposed (S^T[tk,tq] = K Q^T) in 128x512 tiles,
    head-pair packed into the PE array via tile_position (contraction=64);
    the two row-tiled matmuls execute concurrently
  - causal: block skip + column trim + multiplicative triangular band mask
    (applied on gpsimd to keep DVE off the critical path)
  - unnormalized softmax: exp on ACT (scale folded), denominator obtained
    by appending a ones-column to V in the P@V matmul (M=65)
  - normalize via DVE reciprocal + gpsimd partition-broadcast + mult
  - output projection on-device (bf16 partials); host sums TP partials

Scheduling: a single global software pipeline over all 160 attention
chunks (80 per batch) with lag-2 P@V emission, so the PE never waits on
the ACT exp of the current chunk.  QKV (next batch) and output-projection
(current batch) matmul chains are injected between chunks by a pacing
scheduler, keeping the PE stream dense (full p-state) while the ACT
stream stays saturated.  The normalize chain is staggered across chunk
positions so no engine queue head ever blocks on a cross-engine latency.
"""

import os
import sys

sys.path.insert(0, "/opt/trn_rl_repo")

import numpy as np
from contextlib import ExitStack

import concourse.bass as bass
import concourse.mybir as mybir
import concourse.tile as tile
from concourse import bacc
from concourse.bass_utils import run_bass_kernel_spmd

# problem shape (hardcoded per contract)
B, T, C = 4, 2048, 1024
H, G = 16, 4
D = C // H  # 64

# per-core
B_LOC = 2        # batches per core
NPAIR = 2        # head pairs per core (4 heads)
P = 128
CC = C // P      # 8 contraction chunks for projections
NT = 512         # tq tile width
TQT = T // NT    # 4 tq tiles
TKC = T // P     # 16 tk chunks

F32 = mybir.dt.float32
BF16 = mybir.dt.bfloat16
ADT = BF16
Exp = mybir.ActivationFunctionType.Exp
MULT = mybir.AluOpType.mult

# attention chunk list for one batch: (j, p, i); i indexes tk chunks
CHUNKS = [
    (j, p_, i) for j in range(TQT) for p_ in range(NPAIR) for i in range(4 * j + 4)
]
NCH = len(CHUNKS)  # 80
# chunk index at which each block (j, p) ends (batch-relative)
BLOCK_END = {}
for _k, (_j, _p, _i) in enumerate(CHUNKS):
    if _i == 4 * _j + 3:
        BLOCK_END[(_j, _p)] = _k


def _build_program():
    nc = bacc.Bacc(None, target_bir_lowering=False)

    xT = nc.dram_tensor("xT", [B_LOC, C, T], ADT, kind="ExternalInput")
    # columns: q pair0 (128) | q pair1 (128) | k (64) | v (64)
    wqkv = nc.dram_tensor("wqkv", [C, 384], ADT, kind="ExternalInput")
    wproj = nc.dram_tensor("wproj", [2 * P, C], ADT, kind="ExternalInput")
    # multiplicative triangular band mask, duplicated for the 2 packed heads
    maskb = nc.dram_tensor("maskb", [P, 2, P], ADT, kind="ExternalInput")
    ident2 = nc.dram_tensor("ident2", [P, 64], ADT, kind="ExternalInput")
    vones = nc.dram_tensor("vones", [P, TKC], ADT, kind="ExternalInput")
    outp = nc.dram_tensor("outp", [B_LOC, T, C], BF16, kind="ExternalOutput")

    with tile.TileContext(nc) as tc:
        with ExitStack() as ctx:
            const = ctx.enter_context(tc.tile_pool(name="const", bufs=1))
            xp = ctx.enter_context(tc.tile_pool(name="xp", bufs=2))
            sb2 = ctx.enter_context(tc.tile_pool(name="sb2", bufs=2))
            small = ctx.enter_context(tc.tile_pool(name="small", bufs=2))
            ppool = ctx.enter_context(tc.tile_pool(name="ppool", bufs=4))
            stg = ctx.enter_context(tc.tile_pool(name="stg", bufs=3))
            ps_st = ctx.enter_context(tc.tile_pool(name="ps_st", bufs=2, space="PSUM"))
            ps_pv = ctx.enter_context(tc.tile_pool(name="ps_pv", bufs=2, space="PSUM"))
            ps_mm = ctx.enter_context(tc.tile_pool(name="ps_mm", bufs=2, space="PSUM"))

            # ---- constants ----
            wqkv_t = const.tile([P, CC, 384], ADT, tag="wqkv")
            for cc in range(CC):
                nc.sync.dma_start(wqkv_t[:, cc, :], wqkv[cc * P : (cc + 1) * P, :])
            wproj_t = const.tile([P, 2, C], ADT, tag="wproj")
            for cc in range(2):
                nc.sync.dma_start(wproj_t[:, cc, :], wproj[cc * P : (cc + 1) * P, :])
            mask_t = const.tile([P, 2, P], ADT, tag="maskb")
            nc.sync.dma_start(mask_t[:], maskb[:])
            id2_t = const.tile([P, 64], ADT, tag="ident2")
            nc.sync.dma_start(id2_t[:], ident2[:])

            states = {}

            def emit_setup(b, chunked):
                xt = xp.tile([P, CC, T], ADT, tag="xt", name=f"xt{b}")
                # one DMA per token tile (n-major): the first QKV part can
                # start after 1/4 of the bytes, and the qSP FIFO stays short
                for n in range(TQT):
                    nc.sync.dma_start(
                        xt[:, :, n * NT : (n + 1) * NT],
                        xT[b, :, n * NT : (n + 1) * NT].rearrange(
                            "(cc p) t -> p cc t", p=P
                        ),
                    )
                q_sb = sb2.tile([P, NPAIR, T], ADT, tag="q", name=f"q{b}")
                kv_sb = sb2.tile([P, TQT, NT], ADT, tag="kv", name=f"kv{b}")
                k_hi = sb2.tile([P, TQT, NT], ADT, tag="khi", name=f"khi{b}")
                v_a = sb2.tile([P, TKC, 65], ADT, tag="va", name=f"va{b}")
                nc.sync.dma_start(v_a[:, :, 64], vones[:])
                o_t = sb2.tile([P, NPAIR, T], ADT, tag="ot", name=f"ot{b}")
                states[b] = dict(xt=xt, q=q_sb, kv=kv_sb, khi=k_hi, va=v_a, ot=o_t)

            # ---- filler units (atomic PE chains) ----
            def qkv_part(b, n, part):
                # part: 0=kv, 1=q pair0, 2=q pair1
                def f():
                    S = states[b]
                    m = {0: 2, 1: 0, 2: 1}[part]
                    pm = ps_mm.tile([P, NT], F32, tag="mm", name=f"pm{b}_{n}_{part}")
                    for cc in range(CC):
                        nc.tensor.matmul(
                            pm[:],
                            wqkv_t[:, cc, m * P : (m + 1) * P],
                            S["xt"][:, cc, n * NT : (n + 1) * NT],
                            start=(cc == 0),
                            stop=(cc == CC - 1),
                        )
                    if m < 2:
                        nc.vector.tensor_copy(S["q"][:, m, n * NT : (n + 1) * NT], pm[:])
                    else:
                        nc.vector.tensor_copy(S["kv"][:, n, :], pm[:])
                        nc.sync.dma_start(S["khi"][64:128, n, :], S["kv"][0:64, n, :])
                return f

            def vt_unit(b, n):
                def f():
                    S = states[b]
                    for i in range(4 * n, 4 * n + 4):
                        pt = ps_mm.tile([P, 64], ADT, tag="mm", name=f"pt{b}_{i}")
                        nc.tensor.transpose(
                            pt[:],
                            S["kv"][64:128, i // 4, (i % 4) * P : (i % 4 + 1) * P],
                            id2_t[64:128, :],
                        )
                        nc.vector.tensor_copy(S["va"][:, i, 0:64], pt[:])
                return f

            def proj_unit(b, t_):
                def f():
                    S = states[b]
                    stage = stg.tile([P, C], BF16, tag="stage", name=f"stage{b}_{t_}")
                    for n2 in range(2):
                        pm = ps_mm.tile([P, NT], F32, tag="mm", name=f"pj{b}_{t_}_{n2}")
                        for cc2 in range(2):
                            nc.tensor.matmul(
                                pm[:],
                                S["ot"][:, cc2, t_ * P : (t_ + 1) * P],
                                wproj_t[:, cc2, n2 * NT : (n2 + 1) * NT],
                                start=(cc2 == 0),
                                stop=(cc2 == 1),
                            )
                        nc.vector.tensor_copy(stage[:, n2 * NT : (n2 + 1) * NT], pm[:])
                    nc.sync.dma_start(outp[b, t_ * P : (t_ + 1) * P, :], stage[:])
                return f

            def setup_unit(b):
                def f():
                    emit_setup(b, chunked=False)
                return f

            # ---- attention chunk pieces ----
            def emit_scores(b, j, p_, i):
                S = states[b]
                diag = i >= 4 * j
                lo = (i - 4 * j) * P if diag else 0
                st = ps_st.tile([P, 2, NT], F32, tag="st", name=f"st{b}_{j}_{p_}_{i}")
                for e in range(2):
                    ksrc = S["kv"] if e == 0 else S["khi"]
                    nc.tensor.matmul(
                        st[:, e, lo:NT],
                        ksrc[64 * e : 64 * e + 64, i // 4, (i % 4) * P : (i % 4 + 1) * P],
                        S["q"][64 * e : 64 * e + 64, p_, j * NT + lo : (j + 1) * NT],
                        start=True,
                        stop=True,
                        tile_position=(64 * e, 0),
                    )
                pexp = ppool.tile(
                    [P, 2, NT], ADT, tag="pexp", name=f"px{b}_{j}_{p_}_{i}"
                )
                nc.scalar.activation(pexp[:, :, lo:NT], st[:, :, lo:NT], Exp, scale=0.125)
                if diag:
                    nc.gpsimd.tensor_tensor(
                        pexp[:, :, lo : lo + P],
                        pexp[:, :, lo : lo + P],
                        mask_t[:],
                        MULT,
                    )
                return pexp, lo

            blocks = {}  # (b, j, p_) -> dict with pv tiles + norm chain state

            def emit_pv(b, j, p_, i, pexp, lo):
                if i == 0:
                    blocks[(b, j, p_)] = {
                        "pv": [
                            ps_pv.tile([P, NT], F32, tag="pv", name=f"pv{b}_{j}_{p_}_{e}")
                            for e in range(2)
                        ]
                    }
                blk = blocks[(b, j, p_)]
                last = i == 4 * j + 3
                for e in range(2):
                    nc.tensor.matmul(
                        blk["pv"][e][0:65, lo:NT],
                        states[b]["va"][:, i, :],
                        pexp[:, e, lo:NT],
                        start=(i == 0),
                        stop=last,
                    )
                return last

            # normalize chain, staggered over chunk positions
            def norm_p1(b, j, p_):
                # right after the block's last PV: drain PSUM + den row to p0.
                # The tiny den DMA goes on the qAct HWDGE ring so it never
                # queues behind bulk x/out transfers on the qSP ring.
                blk = blocks[(b, j, p_)]
                pvs = small.tile(
                    [65, 2, NT], F32, tag="pvs", bufs=4, name=f"pvs{b}_{j}_{p_}"
                )
                for e in range(2):
                    nc.vector.tensor_copy(pvs[:, e, :], blk["pv"][e][0:65, :])
                blk["pvs"] = pvs
                l0 = small.tile([1, 2, NT], F32, tag="l0", name=f"l0_{b}_{j}_{p_}")
                nc.scalar.dma_start(l0[:], pvs[64:65, :, :])
                blk["l0"] = l0

            def norm_p2(b, j, p_):
                blk = blocks[(b, j, p_)]
                rec0 = small.tile([1, 2, NT], F32, tag="rec0", name=f"rc{b}_{j}_{p_}")
                nc.vector.reciprocal_approx_fast(rec0[:], blk["l0"][:])
                blk["rec0"] = rec0

            def norm_p3(b, j, p_):
                blk = blocks[(b, j, p_)]
                bca = small.tile([64, 2, NT], F32, tag="bca", name=f"bc{b}_{j}_{p_}")
                nc.gpsimd.partition_broadcast(bca[:], blk["rec0"][:])
                blk["bca"] = bca

            def norm_p4(b, j, p_):
                blk = blocks.pop((b, j, p_))
                S = states[b]
                nc.vector.tensor_tensor(
                    S["ot"][0:64, p_, j * NT : (j + 1) * NT],
                    blk["pvs"][0:64, 0, :],
                    blk["bca"][:, 0, :],
                    MULT,
                )
                otmp = small.tile([64, NT], ADT, tag="otmp", name=f"om{b}_{j}_{p_}")
                nc.vector.tensor_tensor(
                    otmp[:], blk["pvs"][0:64, 1, :], blk["bca"][:, 1, :], MULT
                )
                nc.scalar.dma_start(S["ot"][64:128, p_, j * NT : (j + 1) * NT], otmp[:])

            # ---- head: batch-0 setup + first QKV tile, solid ----
            emit_setup(0, chunked=True)
            qkv_part(0, 0, 0)()
            vt_unit(0, 0)()
            qkv_part(0, 0, 1)()
            qkv_part(0, 0, 2)()

            # ---- filler list: (gate, deadline, cost_ns, fn) ----
            QKV_COST = 1700
            VT_COST = 250
            PROJ_COST = 900
            fillers = []

            def add(gate, deadline, cost, fn):
                fillers.append([gate, deadline, cost, fn])

            # batch-0 remaining QKV
            for n in (1, 2, 3):
                dl = {1: 6, 2: 22, 3: 46}[n]
                add(3 * (n - 1), dl, QKV_COST, qkv_part(0, n, 0))
                add(3 * (n - 1), dl + 2, VT_COST, vt_unit(0, n))
                add(3 * (n - 1), dl, QKV_COST, qkv_part(0, n, 1))
                add(3 * (n - 1), {1: 6, 2: 34, 3: 62}[n], QKV_COST, qkv_part(0, n, 2))
            # batch-1 setup (DMA only) + QKV
            add(8, 70, 0, setup_unit(1))
            for n in range(4):
                gate = 24 + 8 * n if n < 2 else 62 + 24 * (n - 2)
                dl = {0: 76, 1: 86, 2: 102, 3: 126}[n]
                add(gate, dl, QKV_COST, qkv_part(1, n, 0))
                add(gate, dl + 2, VT_COST, vt_unit(1, n))
                add(gate, dl, QKV_COST, qkv_part(1, n, 1))
                add(gate, {0: 80, 1: 94, 2: 110, 3: 142}[n], QKV_COST, qkv_part(1, n, 2))
            # projections: gated on the normalize of block (t//4, 1)
            for b in range(2):
                for t_ in range(16):
                    gate = b * NCH + BLOCK_END[(t_ // 4, 1)] + 8
                    dl = 150 if b == 0 else 176
                    add(gate, dl, PROJ_COST, proj_unit(b, t_))
            fillers.sort(key=lambda x: x[0])

            # ---- main pipeline ----
            END = 2 * NCH + 14
            pvq = []
            staggered = {}
            total_cost = sum(f[2] for f in fillers)
            spent = 0.0

            for k in range(END):
                boundary = False
                if k < 2 * NCH:
                    b = k // NCH
                    j, p_, i = CHUNKS[k % NCH]
                    pexp, lo = emit_scores(b, j, p_, i)
                    pvq.append((b, j, p_, i, pexp, lo))
                # lag-2 PV
                if len(pvq) > 2 or (k >= 2 * NCH and pvq):
                    pb, pj, pp, pi, ppx, plo = pvq.pop(0)
                    if emit_pv(pb, pj, pp, pi, ppx, plo):
                        norm_p1(pb, pj, pp)
                        staggered.setdefault(k + 3, []).append(
                            lambda pb=pb, pj=pj, pp=pp: norm_p2(pb, pj, pp)
                        )
                        staggered.setdefault(k + 4, []).append(
                            lambda pb=pb, pj=pj, pp=pp: norm_p3(pb, pj, pp)
                        )
                        staggered.setdefault(k + 6, []).append(
                            lambda pb=pb, pj=pj, pp=pp: norm_p4(pb, pj, pp)
                        )
                        boundary = True
                for fn in staggered.pop(k, []):
                    fn()
                # paced filler injection (skip at block boundaries so the
                # PSUM-draining pvs copies stay at the DVE queue head)
                target = total_cost * (k + 1) / END
                while fillers:
                    g, dl, cost, fn = fillers[0]
                    urgent = dl <= k + 2
                    if not urgent and (g > k or boundary or spent + cost > target):
                        break
                    fillers.pop(0)
                    fn()
                    spent += cost
            for _, _, _, fn in fillers:
                fn()

    nc.compile()
    return nc


_NC = None


def _get_program():
    global _NC
    if _NC is None:
        _NC = _build_program()
    return _NC


def _host_inputs(x, Wq, Wkv, Wproj):
    """Shard + lay out inputs for the 8 cores."""
    import ml_dtypes

    adt_np = ml_dtypes.bfloat16
    tri = np.where(
        np.arange(P)[:, None] <= np.arange(P)[None, :], 1.0, 0.0
    ).astype(np.float32)
    ident2 = np.concatenate([np.eye(64, dtype=np.float32)] * 2, axis=0).astype(
        adt_np
    )  # [128, 64]
    maskb = np.stack([tri, tri], axis=1).astype(adt_np)  # [128, 2, 128]

    in_maps = []
    for d in range(2):
        xT = x[2 * d : 2 * d + 2].transpose(0, 2, 1).astype(adt_np)
        for g in range(G):
            heads = [g, g + 4, g + 8, g + 12]
            wq_cols = np.concatenate(
                [Wq[h * D : (h + 1) * D, :] for h in heads], axis=0
            ).T  # [1024, 256]
            wk = Wkv[g * D : (g + 1) * D, :].T  # [1024, 64]
            wv = Wkv[G * D + g * D : G * D + (g + 1) * D, :].T
            wqkv = np.concatenate([wq_cols, wk, wv], axis=1).astype(adt_np)
            ch = np.concatenate(
                [np.arange(h * D, (h + 1) * D) for h in heads]
            )
            wproj_s = np.ascontiguousarray(Wproj[:, ch].T).astype(adt_np)
            in_maps.append(
                {
                    "xT": xT,
                    "wqkv": wqkv,
                    "wproj": wproj_s,
                    "maskb": maskb,
                    "ident2": ident2,
                    "vones": np.ones((P, TKC), dtype=adt_np),
                }
            )
    return in_maps


def kernel(x, Wq, Wkv, Wproj, b_proj):
    x = np.asarray(x, dtype=np.float32)
    Wq = np.asarray(Wq, dtype=np.float32)
    Wkv = np.asarray(Wkv, dtype=np.float32)
    Wproj = np.asarray(Wproj, dtype=np.float32)
    b_proj = np.asarray(b_proj, dtype=np.float32)

    nc = _get_program()
    in_maps = _host_inputs(x, Wq, Wkv, Wproj)
    trace = bool(int(os.environ.get("BASS_KERNEL_TRACE", "0")))
    res = run_bass_kernel_spmd(nc, in_maps, list(range(8)), trace=trace)
    if trace:
        kernel.last_results = res

    out = np.empty((B, T, C), dtype=np.float32)
    for d in range(2):
        acc = res.results[4 * d]["outp"].astype(np.float32)
        for g in range(1, G):
            acc = acc + res.results[4 * d + g]["outp"].astype(np.float32)
        out[2 * d : 2 * d + 2] = acc + b_proj[None, None, :]
    return out


# revision 11
# speedup vs baseline: 1.6226x; 1.6226x over previous
"""Causal GQA self-attention block (B=4, T=2048, C=1024, H=16, G=4) on 8
Trainium2 NeuronCores.

Sharding: core c = d*4+g  (d in {0,1} batch-DP, g in {0..3} kv-group TP).
Each core handles batches [2d, 2d+1], heads {g, g+4, g+8, g+12}, kv group g,
and produces a partial projection output; the host sums the 4 TP partials
per batch pair and adds the bias.

Per-core kernel (all matmuls bf16, fp32 PSUM accumulation):
  - fused QKV projection from pre-transposed x (host supplies x^T),
    producing Q^T / K^T / V^T with channels on partitions
  - scores computed transposed (S^T[tk,tq] = K Q^T) in 128x512 tiles,
    head-pair packed into the PE array via tile_position (contraction=64);
    the two row-tiled matmuls execute concurrently
  - causal: block skip + column trim + multiplicative triangular band mask
    (applied on gpsimd to keep DVE off the critical path)
  - unnormalized softmax: exp on ACT (scale folded), denominator obtained
    by appending a ones-column to V in the P@V matmul (M=65)
  - normalize via DVE reciprocal + gpsimd partition-broadcast + mult
  - output projection on-device (bf16 partials); host sums TP partials

Scheduling: a single global software pipeline over all 160 attention
chunks (80 per batch) with lag-2 P@V emission, so the PE never waits on
the ACT exp of the current chunk.  QKV (next batch) and output-projection
(current batch) matmul chains are injected between chunks by a pacing
scheduler, keeping the PE stream dense (full p-state) while the ACT
stream stays saturated.  The normalize chain is staggered across chunk
positions so no engine queue head ever blocks on a cross-engine latency.
"""

import os
import sys

sys.path.insert(0, "/opt/trn_rl_repo")

import numpy as np
from contextlib import ExitStack

import concourse.bass as bass
import concourse.mybir as mybir
import concourse.tile as tile
from concourse import bacc
from concourse.bass_utils import run_bass_kernel_spmd

# problem shape (hardcoded per contract)
B, T, C = 4, 2048, 1024
H, G = 16, 4
D = C // H  # 64

# per-core
B_LOC = 2        # batches per core
NPAIR = 2        # head pairs per core (4 heads)
P = 128
CC = C // P      # 8 contraction chunks for projections
NT = 512         # tq tile width
TQT = T // NT    # 4 tq tiles
TKC = T // P     # 16 tk chunks

F32 = mybir.dt.float32
BF16 = mybir.dt.bfloat16
ADT = BF16
Exp = mybir.ActivationFunctionType.Exp
MULT = mybir.AluOpType.mult

# attention chunk list for one batch: (j, p, i); i indexes tk chunks
CHUNKS = [
    (j, p_, i) for j in range(TQT) for p_ in range(NPAIR) for i in range(4 * j + 4)
]
NCH = len(CHUNKS)  # 80
# chunk index at which each block (j, p) ends (batch-relative)
BLOCK_END = {}
for _k, (_j, _p, _i) in enumerate(CHUNKS):
    if _i == 4 * _j + 3:
        BLOCK_END[(_j, _p)] = _k


def _build_program():
    nc = bacc.Bacc(None, target_bir_lowering=False)

    xT = nc.dram_tensor("xT", [B_LOC, C, T], ADT, kind="ExternalInput")
    # columns: q pair0 (128) | q pair1 (128) | k (64) | v (64)
    wqkv = nc.dram_tensor("wqkv", [C, 384], ADT, kind="ExternalInput")
    wproj = nc.dram_tensor("wproj", [2 * P, C], ADT, kind="ExternalInput")
    # multiplicative triangular band mask, duplicated for the 2 packed heads
    maskb = nc.dram_tensor("maskb", [P, 2, P], ADT, kind="ExternalInput")
    ident2 = nc.dram_tensor("ident2", [P, 64], ADT, kind="ExternalInput")
    vones = nc.dram_tensor("vones", [P, TKC], ADT, kind="ExternalInput")
    outp = nc.dram_tensor("outp", [B_LOC, T, C], BF16, kind="ExternalOutput")

    with tile.TileContext(nc) as tc:
        with ExitStack() as ctx:
            const = ctx.enter_context(tc.tile_pool(name="const", bufs=1))
            xp = ctx.enter_context(tc.tile_pool(name="xp", bufs=2))
            sb2 = ctx.enter_context(tc.tile_pool(name="sb2", bufs=2))
            small = ctx.enter_context(tc.tile_pool(name="small", bufs=2))
            ppool = ctx.enter_context(tc.tile_pool(name="ppool", bufs=4))
            stg = ctx.enter_context(tc.tile_pool(name="stg", bufs=3))
            ps_st = ctx.enter_context(tc.tile_pool(name="ps_st", bufs=2, space="PSUM"))
            ps_pv = ctx.enter_context(tc.tile_pool(name="ps_pv", bufs=2, space="PSUM"))
            ps_mm = ctx.enter_context(tc.tile_pool(name="ps_mm", bufs=2, space="PSUM"))

            # ---- constants ----
            wqkv_t = const.tile([P, CC, 384], ADT, tag="wqkv")
            for cc in range(CC):
                nc.sync.dma_start(wqkv_t[:, cc, :], wqkv[cc * P : (cc + 1) * P, :])
            wproj_t = const.tile([P, 2, C], ADT, tag="wproj")
            for cc in range(2):
                nc.sync.dma_start(wproj_t[:, cc, :], wproj[cc * P : (cc + 1) * P, :])
            mask_t = const.tile([P, 2, P], ADT, tag="maskb")
            nc.sync.dma_start(mask_t[:], maskb[:])
            id2_t = const.tile([P, 64], ADT, tag="ident2")
            nc.sync.dma_start(id2_t[:], ident2[:])

            states = {}

            def emit_setup(b, chunked):
                xt = xp.tile([P, CC, T], ADT, tag="xt", name=f"xt{b}")
                # one DMA per token tile (n-major): the first QKV part can
                # start after 1/4 of the bytes, and the qSP FIFO stays short
                for n in range(TQT):
                    nc.sync.dma_start(
                        xt[:, :, n * NT : (n + 1) * NT],
                        xT[b, :, n * NT : (n + 1) * NT].rearrange(
                            "(cc p) t -> p cc t", p=P
                        ),
                    )
                q_sb = sb2.tile([P, NPAIR, T], ADT, tag="q", name=f"q{b}")
                kv_sb = sb2.tile([P, TQT, NT], ADT, tag="kv", name=f"kv{b}")
                k_hi = sb2.tile([P, TQT, NT], ADT, tag="khi", name=f"khi{b}")
                v_a = sb2.tile([P, TKC, 65], ADT, tag="va", name=f"va{b}")
                nc.sync.dma_start(v_a[:, :, 64], vones[:])
                o_t = sb2.tile([P, NPAIR, T], ADT, tag="ot", name=f"ot{b}")
                states[b] = dict(xt=xt, q=q_sb, kv=kv_sb, khi=k_hi, va=v_a, ot=o_t)

            # ---- filler units (atomic PE chains) ----
            def qkv_part(b, n, part):
                # part: 0=kv, 1=q pair0, 2=q pair1
                def f():
                    S = states[b]
                    m = {0: 2, 1: 0, 2: 1}[part]
                    pm = ps_mm.tile([P, NT], F32, tag="mm", name=f"pm{b}_{n}_{part}")
                    for cc in range(CC):
                        nc.tensor.matmul(
                            pm[:],
                            wqkv_t[:, cc, m * P : (m + 1) * P],
                            S["xt"][:, cc, n * NT : (n + 1) * NT],
                            start=(cc == 0),
                            stop=(cc == CC - 1),
                        )
                    if m < 2:
                        nc.vector.tensor_copy(S["q"][:, m, n * NT : (n + 1) * NT], pm[:])
                    else:
                        nc.vector.tensor_copy(S["kv"][:, n, :], pm[:])
                        nc.sync.dma_start(S["khi"][64:128, n, :], S["kv"][0:64, n, :])
                return f

            def vt_unit(b, n):
                def f():
                    S = states[b]
                    for i in range(4 * n, 4 * n + 4):
                        pt = ps_mm.tile([P, 64], ADT, tag="mm", name=f"pt{b}_{i}")
                        nc.tensor.transpose(
                            pt[:],
                            S["kv"][64:128, i // 4, (i % 4) * P : (i % 4 + 1) * P],
                            id2_t[64:128, :],
                        )
                        nc.vector.tensor_copy(S["va"][:, i, 0:64], pt[:])
                return f

            def proj_unit(b, t_):
                def f():
                    S = states[b]
                    stage = stg.tile([P, C], BF16, tag="stage", name=f"stage{b}_{t_}")
                    for n2 in range(2):
                        pm = ps_mm.tile([P, NT], F32, tag="mm", name=f"pj{b}_{t_}_{n2}")
                        for cc2 in range(2):
                            nc.tensor.matmul(
                                pm[:],
                                S["ot"][:, cc2, t_ * P : (t_ + 1) * P],
                                wproj_t[:, cc2, n2 * NT : (n2 + 1) * NT],
                                start=(cc2 == 0),
                                stop=(cc2 == 1),
                            )
                        nc.vector.tensor_copy(stage[:, n2 * NT : (n2 + 1) * NT], pm[:])
                    nc.sync.dma_start(outp[b, t_ * P : (t_ + 1) * P, :], stage[:])
                return f

            def setup_unit(b):
                def f():
                    emit_setup(b, chunked=False)
                return f

            # ---- attention chunk pieces ----
            def emit_scores(b, j, p_, i):
                S = states[b]
                diag = i >= 4 * j
                lo = (i - 4 * j) * P if diag else 0
                st = ps_st.tile([P, 2, NT], F32, tag="st", name=f"st{b}_{j}_{p_}_{i}")
                for e in range(2):
                    ksrc = S["kv"] if e == 0 else S["khi"]
                    nc.tensor.matmul(
                        st[:, e, lo:NT],
                        ksrc[64 * e : 64 * e + 64, i // 4, (i % 4) * P : (i % 4 + 1) * P],
                        S["q"][64 * e : 64 * e + 64, p_, j * NT + lo : (j + 1) * NT],
                        start=True,
                        stop=True,
                        tile_position=(64 * e, 0),
                    )
                pexp = ppool.tile(
                    [P, 2, NT], ADT, tag="pexp", name=f"px{b}_{j}_{p_}_{i}"
                )
                nc.scalar.activation(pexp[:, :, lo:NT], st[:, :, lo:NT], Exp, scale=0.125)
                if diag:
                    nc.vector.tensor_tensor(
                        pexp[:, :, lo : lo + P],
                        pexp[:, :, lo : lo + P],
                        mask_t[:],
                        MULT,
                    )
                return pexp, lo

            blocks = {}  # (b, j, p_) -> dict with pv tiles + norm chain state

            def emit_pv(b, j, p_, i, pexp, lo):
                if i == 0:
                    blocks[(b, j, p_)] = {
                        "pv": [
                            ps_pv.tile([P, NT], F32, tag="pv", name=f"pv{b}_{j}_{p_}_{e}")
                            for e in range(2)
                        ]
                    }
                blk = blocks[(b, j, p_)]
                last = i == 4 * j + 3
                for e in range(2):
                    nc.tensor.matmul(
                        blk["pv"][e][0:65, lo:NT],
                        states[b]["va"][:, i, :],
                        pexp[:, e, lo:NT],
                        start=(i == 0),
                        stop=last,
                    )
                return last

            # normalize chain, staggered over chunk positions
            def norm_p1(b, j, p_):
                # right after the block's last PV: drain PSUM + den row to p0.
                # The tiny den DMA goes on the qAct HWDGE ring so it never
                # queues behind bulk x/out transfers on the qSP ring.
                blk = blocks[(b, j, p_)]
                pvs = small.tile(
                    [65, 2, NT], F32, tag="pvs", bufs=4, name=f"pvs{b}_{j}_{p_}"
                )
                for e in range(2):
                    nc.vector.tensor_copy(pvs[:, e, :], blk["pv"][e][0:65, :])
                blk["pvs"] = pvs
                l0 = small.tile([1, 2, NT], F32, tag="l0", name=f"l0_{b}_{j}_{p_}")
                nc.scalar.dma_start(l0[:], pvs[64:65, :, :])
                blk["l0"] = l0

            def norm_p2(b, j, p_):
                blk = blocks[(b, j, p_)]
                rec0 = small.tile([1, 2, NT], F32, tag="rec0", name=f"rc{b}_{j}_{p_}")
                nc.vector.reciprocal_approx_fast(rec0[:], blk["l0"][:])
                blk["rec0"] = rec0

            def norm_p3(b, j, p_):
                blk = blocks[(b, j, p_)]
                bca = small.tile([64, 2, NT], F32, tag="bca", name=f"bc{b}_{j}_{p_}")
                nc.gpsimd.partition_broadcast(bca[:], blk["rec0"][:])
                blk["bca"] = bca

            def norm_p4(b, j, p_):
                blk = blocks.pop((b, j, p_))
                S = states[b]
                nc.vector.tensor_tensor(
                    S["ot"][0:64, p_, j * NT : (j + 1) * NT],
                    blk["pvs"][0:64, 0, :],
                    blk["bca"][:, 0, :],
                    MULT,
                )
                otmp = small.tile([64, NT], ADT, tag="otmp", name=f"om{b}_{j}_{p_}")
                nc.vector.tensor_tensor(
                    otmp[:], blk["pvs"][0:64, 1, :], blk["bca"][:, 1, :], MULT
                )
                nc.sync.dma_start(S["ot"][64:128, p_, j * NT : (j + 1) * NT], otmp[:])

            # ---- head: batch-0 setup + first QKV tile, solid ----
            emit_setup(0, chunked=True)
            qkv_part(0, 0, 0)()
            vt_unit(0, 0)()
            qkv_part(0, 0, 1)()
            qkv_part(0, 0, 2)()

            # ---- filler list: (gate, deadline, cost_ns, fn) ----
            QKV_COST = 1700
            VT_COST = 250
            PROJ_COST = 900
            fillers = []

            def add(gate, deadline, cost, fn):
                fillers.append([gate, deadline, cost, fn])

            # batch-0 remaining QKV
            for n in (1, 2, 3):
                dl = {1: 6, 2: 22, 3: 46}[n]
                add(3 * (n - 1), dl, QKV_COST, qkv_part(0, n, 0))
                add(3 * (n - 1), dl + 2, VT_COST, vt_unit(0, n))
                add(3 * (n - 1), dl, QKV_COST, qkv_part(0, n, 1))
                add(3 * (n - 1), {1: 6, 2: 34, 3: 62}[n], QKV_COST, qkv_part(0, n, 2))
            # batch-1 setup (DMA only) + QKV
            add(8, 70, 0, setup_unit(1))
            for n in range(4):
                gate = 24 + 8 * n if n < 2 else 62 + 24 * (n - 2)
                dl = {0: 76, 1: 86, 2: 102, 3: 126}[n]
                add(gate, dl, QKV_COST, qkv_part(1, n, 0))
                add(gate, dl + 2, VT_COST, vt_unit(1, n))
                add(gate, dl, QKV_COST, qkv_part(1, n, 1))
                add(gate, {0: 80, 1: 94, 2: 110, 3: 142}[n], QKV_COST, qkv_part(1, n, 2))
            # projections: gated on the normalize of block (t//4, 1)
            for b in range(2):
                for t_ in range(16):
                    gate = b * NCH + BLOCK_END[(t_ // 4, 1)] + 8
                    dl = 150 if b == 0 else 176
                    add(gate, dl, PROJ_COST, proj_unit(b, t_))
            fillers.sort(key=lambda x: x[0])

            # ---- main pipeline ----
            END = 2 * NCH + 14
            pvq = []
            staggered = {}
            total_cost = sum(f[2] for f in fillers)
            spent = 0.0

            for k in range(END):
                boundary = False
                if k < 2 * NCH:
                    b = k // NCH
                    j, p_, i = CHUNKS[k % NCH]
                    pexp, lo = emit_scores(b, j, p_, i)
                    pvq.append((b, j, p_, i, pexp, lo))
                # lag-2 PV
                if len(pvq) > 2 or (k >= 2 * NCH and pvq):
                    pb, pj, pp, pi, ppx, plo = pvq.pop(0)
                    if emit_pv(pb, pj, pp, pi, ppx, plo):
                        norm_p1(pb, pj, pp)
                        staggered.setdefault(k + 3, []).append(
                            lambda pb=pb, pj=pj, pp=pp: norm_p2(pb, pj, pp)
                        )
                        staggered.setdefault(k + 4, []).append(
                            lambda pb=pb, pj=pj, pp=pp: norm_p3(pb, pj, pp)
                        )
                        staggered.setdefault(k + 6, []).append(
                            lambda pb=pb, pj=pj, pp=pp: norm_p4(pb, pj, pp)
                        )
                        boundary = True
                for fn in staggered.pop(k, []):
                    fn()
                # paced filler injection (skip at block boundaries so the
                # PSUM-draining pvs copies stay at the DVE queue head)
                target = total_cost * (k + 1) / END
                while fillers:
                    g, dl, cost, fn = fillers[0]
                    urgent = dl <= k + 2
                    if not urgent and (g > k or boundary or spent + cost > target):
                        break
                    fillers.pop(0)
                    fn()
                    spent += cost
            for _, _, _, fn in fillers:
                fn()

    nc.compile()
    return nc


_NC = None


def _get_program():
    global _NC
    if _NC is None:
        _NC = _build_program()
    return _NC


def _host_inputs(x, Wq, Wkv, Wproj):
    """Shard + lay out inputs for the 8 cores."""
    import ml_dtypes

    adt_np = ml_dtypes.bfloat16
    tri = np.where(
        np.arange(P)[:, None] <= np.arange(P)[None, :], 1.0, 0.0
    ).astype(np.float32)
    ident2 = np.concatenate([np.eye(64, dtype=np.float32)] * 2, axis=0).astype(
        adt_np
    )  # [128, 64]
    maskb = np.stack([tri, tri], axis=1).astype(adt_np)  # [128, 2, 128]

    in_maps = []
    for d in range(2):
        xT = x[2 * d : 2 * d + 2].transpose(0, 2, 1).astype(adt_np)
        for g in range(G):
            heads = [g, g + 4, g + 8, g + 12]
            wq_cols = np.concatenate(
                [Wq[h * D : (h + 1) * D, :] for h in heads], axis=0
            ).T  # [1024, 256]
            wk = Wkv[g * D : (g + 1) * D, :].T  # [1024, 64]
            wv = Wkv[G * D + g * D : G * D + (g + 1) * D, :].T
            wqkv = np.concatenate([wq_cols, wk, wv], axis=1).astype(adt_np)
            ch = np.concatenate(
                [np.arange(h * D, (h + 1) * D) for h in heads]
            )
            wproj_s = np.ascontiguousarray(Wproj[:, ch].T).astype(adt_np)
            in_maps.append(
                {
                    "xT": xT,
                    "wqkv": wqkv,
                    "wproj": wproj_s,
                    "maskb": maskb,
                    "ident2": ident2,
                    "vones": np.ones((P, TKC), dtype=adt_np),
                }
            )
    return in_maps


def kernel(x, Wq, Wkv, Wproj, b_proj):
    x = np.asarray(x, dtype=np.float32)
    Wq = np.asarray(Wq, dtype=np.float32)
    Wkv = np.asarray(Wkv, dtype=np.float32)
    Wproj = np.asarray(Wproj, dtype=np.float32)
    b_proj = np.asarray(b_proj, dtype=np.float32)

    nc = _get_program()
    in_maps = _host_inputs(x, Wq, Wkv, Wproj)
    trace = bool(int(os.environ.get("BASS_KERNEL_TRACE", "0")))
    res = run_bass_kernel_spmd(nc, in_maps, list(range(8)), trace=trace)
    if trace:
        kernel.last_results = res

    out = np.empty((B, T, C), dtype=np.float32)
    for d in range(2):
        acc = res.results[4 * d]["outp"].astype(np.float32)
        for g in range(1, G):
            acc = acc + res.results[4 * d + g]["outp"].astype(np.float32)
        out[2 * d : 2 * d + 2] = acc + b_proj[None, None, :]
    return out
